# revision 24
# baseline (speedup 1.0000x reference)
"""Trainium2 Bass kernel for nn_CrossEntropyLossWeight3.

Math: per row b of predict/target [B,16]:
  probs   = softmax(predict[b])
  pre     = argmax(predict[b]);  tar = argmax(target[b])
  w       = 0 if pre==tar else penalty[tar, pre]
  loss_b  = w * probs[pre]
out = mean_b(loss_b)

Key identities used on-device:
  probs[pre]   = exp(max(x)) / sum(exp(x))      (softmax at its own argmax)
  penalty[i,j] = max(c_i,c_j)/(c_i+c_j) with distinct per-class counts c;
  with u = c[pre], v = c[tar]:  w = (u != v) * max(u,v)/(u+v).
  counts/1000 (9 bits, exact) are embedded into the low mantissa bits of the
  raw inputs, so one fused embed+segmented-max DVE scan per tensor yields
  the row max together with its argmax's class count (<= 2^-14 relative
  perturbation). Two more fused custom DVE ops evaluate the whole per-row
  weight formula straight from the embedded maxima:
    WNUM = (u!=v) * max(u,v)        SPD = u + v
  so loss_b = WNUM * exp(m) / (SPD * sumexp).

v6 engine balance (per [128, 256*16] tile; single sync HWDGE ring streams
both tensors at a measured ~428 GB/s => ~9.4us/tile of DMA):
  - DVE     : two embed+segmax f32 scans (2 x 4.4us) + WNUM/SPD/recip per
              tile (~1.3us)  => ~10.1us/tile, the critical engine
  - ACT     : exp(predict) f32->bf16 (3.7us) + exp(m) (~0.3us)
  - TensorE : row sums of E as 16 PSUM-accumulated matmuls with identity
              weights (rhs = E[:, :, w], w=0..15) -> s[p,r] lands in PSUM
              in f32, ~2-3us/tile on an otherwise idle engine
  - GPSIMD  : only the small per-tile formula mults den/num/num2/acc
              (Q7 is ~2x slower under full DMA load; it gets no streaming
              work at all)
  - DMA     : both input streams + out on the SP (sync) ring so ACT's exp
              never sits in front of a dma_start issue
  - formula : per tile, split into F1 (wn/sp/em/den/num, emitted with the
              tile) and F2 (rec/num2/acc, deferred one tile) so no engine
              head-of-line stalls on a cross-engine dependency
Sharding: pure data parallel over 8 cores (batch split); each core returns
per-partition partial sums [128,256]; host reduces and divides by B.
"""

import sys

sys.path.insert(0, "/opt/trn_rl_repo")

import numpy as np

import concourse.bass as bass
import concourse.bacc as bacc
import concourse.tile as tile
from concourse import mybir
from concourse.bass_utils import run_bass_kernel_spmd

B, W = 2097152, 16
NCORES = 8
BS = B // NCORES          # rows per core
P = 128                   # SBUF partitions
R = 256                   # rows per partition per tile
F = R * W                 # free elems per partition per tile
TILE_ROWS = P * R
NT = BS // TILE_ROWS      # tiles per core

LABELS_NUM_COUNT = [500000, 120000, 80000, 45000, 30000, 250000, 15000, 9000,
                    60000, 7000, 180000, 22000, 11000, 95000, 5000, 40000]

f32 = mybir.dt.float32
bf16 = mybir.dt.bfloat16
u32 = mybir.dt.uint32
AX = mybir.AxisListType
OP = mybir.AluOpType
ACT = mybir.ActivationFunctionType

PAYLOAD_BITS = 9          # counts/1000 <= 500 fits in 9 bits exactly
PAYLOAD_MASK = (1 << PAYLOAD_BITS) - 1
F_2P23 = 8388608.0        # bit pattern 0x4B000000; OR'ing these bits onto the
                          # 9-bit payload makes the exact float 2^23 + payload
F_2P24 = 16777216.0


def _register_custom_ops():
    """Three runtime-registered custom DVE ops.

    EMBMAX_SEG_ANT: fused "embed payload + segmented max" scan (see v2/v3
      history): body = Scan(MAX, ((x|c)^c)|pay, _subdim_step=Zero) over a
      [P, S, 16] view; stride-0 out leaves per-segment maxima in [P, S].
      The OR/XOR form avoids an AND with 0xFFFFFE00 (NaN bit pattern).
    WNUM_ANT(me, mt; s0=mask, s1=2^23): with u' = (me & mask) | bits(s1),
      v' = (mt & mask) | bits(s1)  (both exact floats 2^23 + count):
      out = (u' != v') * (max(u',v') - 2^23) = (u!=v)*max(u,v).
    SPD_ANT(me, mt; s0, s1, imm2=2^24): out = u' + v' - 2^24 = u + v.
    """
    import numpy as np_

    from concourse.dve_spec import (
        Spec, Src0, Src1, C0, C1, C2, Bin, AluOp, lower, ne, maxx, Zero,
    )
    from concourse.dve_ops import (
        DveOp,
        OPS,
        CUSTOM_DVE_SPECS,
        _SUB_OPCODE_FOR_NAME,
        _CUSTOM_DVE_ROW_BASE,
        _COMPILE_CACHE,
    )
    from concourse.dve_uop import DveOpSpec
    import concourse.dve_spec as ds

    def reg(name, spec, rd1):
        for o in OPS:
            if o.name == name:
                return o
        shas = {}
        for ver in ("v3", "v4"):
            uops = lower(spec, ver=ver)
            s = DveOpSpec(
                name=name,
                opcode=_CUSTOM_DVE_ROW_BASE + len(OPS),
                uops=uops,
                rd1_en=rd1,
            )
            shas[ver] = s.sha(ver)
        op = DveOp(name, spec, subdim=False, uops_sha=shas)
        _SUB_OPCODE_FOR_NAME[name] = _CUSTOM_DVE_ROW_BASE + len(OPS)
        OPS.append(op)
        CUSTOM_DVE_SPECS[name] = spec
        return op

    embed_expr = Bin(
        AluOp.BITWISE_OR,
        Bin(AluOp.BITWISE_XOR, Bin(AluOp.BITWISE_OR, Src0, C0), C0),
        Src1,
    )

    def _ref_embmax(in0, in1, s0, s1, imm2):
        emb = (
            ((in0.view(np_.uint32) | PAYLOAD_MASK) ^ PAYLOAD_MASK)
            | in1.view(np_.uint32)
        ).view(np_.float32)
        return np_.maximum.accumulate(emb, axis=-1)

    def reg_embmax():
        name = "EMBMAX_SEG_ANT"
        for o in OPS:
            if o.name == name:
                return o
        seg = ds.Scan(op=AluOp.MAX, expr=embed_expr, init=None, _subdim_step=Zero)
        spec = Spec(body=seg, reference=_ref_embmax)
        orig_so, orig_nas = ds._scan_overrides, ds._node_as_stage

        def patched_so(scans, node_stage):
            seed, step = {}, {}
            for scan in scans:
                d = node_stage[scan]
                init = (
                    scan.init
                    if scan.init is not None
                    else ds._ACCUM_IDENTITY[scan.op]
                )
                seed[d] = orig_nas(init)
                if scan._subdim_step is not None:
                    step[d] = ds._Stage(AluOp.BYPASS, scan.expr)
            return seed, step

        def patched_nas(e):
            if isinstance(e, ds.Scan) and e._subdim_step is not None:
                return ds._Stage(e.op, ds.AluInp.CURR_ALU_OUT, e.expr)
            return orig_nas(e)

        uops_by_ver, shas = {}, {}
        ds._scan_overrides, ds._node_as_stage = patched_so, patched_nas
        try:
            for ver in ("v3", "v4"):
                uops_by_ver[ver] = lower(spec, ver=ver)
        finally:
            ds._scan_overrides, ds._node_as_stage = orig_so, orig_nas
        opcode = _CUSTOM_DVE_ROW_BASE + len(OPS)
        for ver in ("v3", "v4"):
            s = DveOpSpec(name=name, opcode=opcode, uops=uops_by_ver[ver], rd1_en=True)
            shas[ver] = s.sha(ver)
            _COMPILE_CACHE[(name, ver)] = s
        op = DveOp(name, spec, subdim=True, uops_sha=shas)
        _SUB_OPCODE_FOR_NAME[name] = opcode
        OPS.append(op)
        CUSTOM_DVE_SPECS[name] = spec
        return op

    def _uprime(src):
        return Bin(AluOp.BITWISE_OR, Bin(AluOp.BITWISE_AND, src, C0), C1)

    def _np_uprime(x):
        return (
            (x.view(np_.uint32) & PAYLOAD_MASK) | np_.uint32(0x4B000000)
        ).view(np_.float32)

    up_e, vp_e = _uprime(Src0), _uprime(Src1)
    wnum_spec = Spec(
        body=Bin(
            AluOp.MULTIPLY,
            ne(up_e, vp_e),
            Bin(AluOp.SUBTRACT, maxx(up_e, vp_e), C1),
        ),
        reference=lambda in0, in1, s0, s1, imm2: np_.where(
            _np_uprime(in0) != _np_uprime(in1),
            np_.maximum(_np_uprime(in0), _np_uprime(in1)) - np_.float32(F_2P23),
            np_.float32(0.0),
        ).astype(np_.float32),
    )
    spd_spec = Spec(
        body=Bin(
            AluOp.SUBTRACT, Bin(AluOp.ADD, up_e, vp_e), C2
        ),
        reference=lambda in0, in1, s0, s1, imm2: (
            _np_uprime(in0) + _np_uprime(in1) - np_.float32(F_2P24)
        ).astype(np_.float32),
    )

    embed = reg_embmax()
    wnum = reg("WNUM_ANT", wnum_spec, rd1=True)
    spd = reg("SPD_ANT", spd_spec, rd1=True)
    return embed, wnum, spd


BW = 2 * R                 # formula block width: one tile PAIR (512)
NBLK = NT // 2             # formula blocks (pairs) per core


def _emit_tile(nc, pools, pred_v, targ_v, pay_b, t, embed_op, mask_ap,
               me, mt, e2):
    """Streaming part for one [128, R*16] tile. Row stats land in column
    half t%2 of the pair stats tiles me/mt; exp lands in half t%2 of the
    bf16 pair tile e2."""
    io_pool = pools[0]
    cols = slice((t % 2) * R, (t % 2 + 1) * R)

    # both input streams on the sync HWDGE ring: SP issues nothing else, so
    # dma_starts go out back-to-back and are never stuck behind an ACT op
    xp = io_pool.tile([P, F], f32, tag="xp")
    nc.sync.dma_start(out=xp[:, :], in_=pred_v[t])
    xt = io_pool.tile([P, F], f32, tag="xt")
    nc.sync.dma_start(out=xt[:, :], in_=targ_v[t])

    # fused embed + segmented max over RAW predict on DVE; runs concurrently
    # with the exp pass on ACT (both only read xp)
    xp3 = xp[:, :].rearrange("p (r w) -> p r w", w=W)
    nc.vector._custom_dve(
        embed_op,
        out=me[:, cols].unsqueeze(2).broadcast_to([P, R, W]),
        in0=xp3, in1=pay_b, s0=mask_ap,
    )

    # E = exp(predict) on ScalarE into half t%2 of the bf16 pair tile
    # (contiguous write — a transposed write ran 5x slower on ACT)
    nc.scalar.activation(e2[:, (t % 2) * F:(t % 2 + 1) * F], xp[:, :],
                         ACT.Exp)

    # target side: fused embed + segmented max on DVE
    xt3 = xt[:, :].rearrange("p (r w) -> p r w", w=W)
    nc.vector._custom_dve(
        embed_op,
        out=mt[:, cols].unsqueeze(2).broadcast_to([P, R, W]),
        in0=xt3, in1=pay_b, s0=mask_ap,
    )


def _emit_sums_pe(nc, pools, e2, half, ident_b):
    """Row sums of one e2 half (one tile) on TensorE: 16 matmuls with
    identity weights, one per class column (strided rhs -> ~2.1ns/col on
    HW), PSUM-accumulated in f32. Issued per tile (right after its exp) so
    PE work spreads across the whole stream. Returns the PSUM tile; the
    ACT drain to SBUF is emitted by the caller one tile LATER, so the next
    exp in ACT program order never waits on this tile's matmuls."""
    ps_pool = pools[4]
    s2 = ps_pool.tile([P, R], f32, tag="s2")
    e3h = e2[:, half * F:(half + 1) * F].rearrange("p (r w) -> p r w", w=W)
    for w in range(W):
        nc.tensor.matmul(
            out=s2[:, :], lhsT=ident_b[:, :], rhs=e3h[:, :, w],
            start=(w == 0), stop=(w == W - 1),
        )
    return s2


def _emit_sums_dve_tree(nc, pools, e2, half, s2c):
    """Row sums of one e2 half via the contiguous-halves bf16 pairwise-add
    tree on DVE. Used ONLY for the final tile: the PE can't start that
    tile's ~10us of matmuls until the stream has already ended, and the
    GPSIMD tree there ran ~13us while halving concurrent DVE scan speed;
    DVE's own tree is ~2.6-4.6us with no cross-engine contention."""
    work_pool = pools[1]
    e3 = e2[:, half * F:(half + 1) * F].rearrange("p (r w) -> p r w", w=W)
    l1 = work_pool.tile([P, R * 8], bf16, tag="l1")
    l1v = l1[:, :].rearrange("p (r h) -> p r h", h=8)
    nc.vector.tensor_tensor(l1v, e3[:, :, 0:8], e3[:, :, 8:16], op=OP.add)
    l2 = work_pool.tile([P, R * 4], bf16, tag="l2")
    l2v = l2[:, :].rearrange("p (r h) -> p r h", h=4)
    nc.vector.tensor_tensor(l2v, l1v[:, :, 0:4], l1v[:, :, 4:8], op=OP.add)
    l3 = work_pool.tile([P, R * 2], bf16, tag="l3")
    l3v = l3[:, :].rearrange("p (r h) -> p r h", h=2)
    nc.vector.tensor_tensor(l3v, l2v[:, :, 0:2], l2v[:, :, 2:4], op=OP.add)
    nc.vector.tensor_tensor(
        s2c[:, half * R:(half + 1) * R].unsqueeze(2),
        l3v[:, :, 0:1], l3v[:, :, 1:2], op=OP.add)


def _emit_f1(nc, pools, me, mt, s2c, ops, mask_ap, last):
    """Formula stage 1 for one [128, BW] pair block (emitted with its odd
    tile): everything that only needs me/mt/s.
      wn = (u!=v)*max(u,v)   sp = u+v   em = exp(m)
      den = sp * sumexp      num = wn * em
    den/num run on GPSIMD (near idle) except for the last block, where
    DVE's ~0.7us ops shorten the post-DMA tail. Returns (den, num)."""
    fp_pool = pools[3]
    _, wnum_op, spd_op = ops
    mul_eng = nc.vector if last else nc.gpsimd

    wn = fp_pool.tile([P, BW], f32, tag="wn")
    nc.vector._custom_dve(wnum_op, out=wn[:, :], in0=me[:, :], in1=mt[:, :],
                          s0=mask_ap, s1=F_2P23)
    sp = fp_pool.tile([P, BW], f32, tag="sp")
    nc.vector._custom_dve(spd_op, out=sp[:, :], in0=me[:, :], in1=mt[:, :],
                          s0=mask_ap, s1=F_2P23, imm2=F_2P24)
    # em = exp(m): payload bits perturb m by <= 2^-14 relative — in budget
    em = fp_pool.tile([P, BW], f32, tag="em")
    nc.scalar.activation(em[:, :], me[:, :], ACT.Exp)

    den = fp_pool.tile([P, BW], f32, tag="dn")
    if last:
        # both s2c halves are already written (tile NT-2 drained during
        # tile NT-1; tile NT-1 via the DVE tree): one full-width den
        mul_eng.tensor_tensor(den[:, :], sp[:, :], s2c[:, :], op=OP.mult)
    else:
        # s2c's second half is only drained from PSUM during the NEXT
        # tile (deferred ACT drain), so den's second half moves to F2
        mul_eng.tensor_tensor(den[:, :R], sp[:, :R], s2c[:, :R],
                              op=OP.mult)
    num = fp_pool.tile([P, BW], f32, tag="nm")
    mul_eng.tensor_tensor(num[:, :], wn[:, :], em[:, :], op=OP.mult)
    return sp, s2c, den, num


def _emit_f2(nc, pools, res_sl, sp, s2c, den, num, last):
    """Formula stage 2 (emitted one tile later so no engine head-of-line
    stalls on a cross-engine dep): finish den, then res = num / den."""
    fp_pool = pools[3]
    if not last:
        nc.gpsimd.tensor_tensor(den[:, R:], sp[:, R:], s2c[:, R:],
                                op=OP.mult)
    rec = fp_pool.tile([P, BW], f32, tag="rc")
    nc.vector.reciprocal_approx_fast(out=rec[:, :], in_=den[:, :])
    mul_eng = nc.vector if last else nc.gpsimd
    mul_eng.tensor_tensor(res_sl, num[:, :], rec[:, :], op=OP.mult)


def _emit_pass(nc, pools, pred_v, targ_v, pay_b, ident_b, res, ops, mask_ap):
    _, work_pool, stats_pool, fp_pool, ps_pool = pools
    embed_op = ops[0]
    pend = None            # (den, num) of the previous pair block
    pend_drain = None      # (psum_tile, s2c, half) not yet drained by ACT
    for k in range(NBLK):
        me = stats_pool.tile([P, BW], f32, tag="me")
        mt = stats_pool.tile([P, BW], f32, tag="mt")
        s2c = fp_pool.tile([P, BW], bf16, tag="s2c")
        e2 = work_pool.tile([P, 2 * F], bf16, tag="e2")
        for sub in range(2):
            t = 2 * k + sub
            _emit_tile(nc, pools, pred_v, targ_v, pay_b, t, embed_op,
                       mask_ap, me, mt, e2)
            # drain the PREVIOUS tile's PSUM sums now — after this tile's
            # exp in ACT program order, so exp never waits on matmuls
            if pend_drain is not None:
                ps, dst, dhalf = pend_drain
                nc.scalar.activation(dst[:, dhalf * R:(dhalf + 1) * R],
                                     ps[:, :], ACT.Copy)
                pend_drain = None
            if t < NT - 1:
                pend_drain = (_emit_sums_pe(nc, pools, e2, sub, ident_b),
                              s2c, sub)
            else:
                _emit_sums_dve_tree(nc, pools, e2, sub, s2c)
            # interleave the previous pair's F2 early in this pair
            if pend is not None and sub == 0:
                _emit_f2(nc, pools, res[:, (k - 1) * BW:k * BW], *pend,
                         last=False)
                pend = None
        pend = _emit_f1(nc, pools, me, mt, s2c, ops, mask_ap,
                        last=(k == NBLK - 1))
    _emit_f2(nc, pools, res[:, (NBLK - 1) * BW:], *pend, last=True)


def _build_program():
    nc = bacc.Bacc("TRN2", target_bir_lowering=False, debug=False)
    pred = nc.dram_tensor("predict", [BS, W], f32, kind="ExternalInput")
    targ = nc.dram_tensor("target", [BS, W], f32, kind="ExternalInput")
    pay = nc.dram_tensor("payload", [P, W], u32, kind="ExternalInput")
    ident = nc.dram_tensor("ident", [P, P], f32, kind="ExternalInput")
    out = nc.dram_tensor("out", [P, NBLK * BW], f32, kind="ExternalOutput")

    pred_v = pred[:, :].rearrange("(t p r) w -> t p (r w)", t=NT, p=P, r=R)
    targ_v = targ[:, :].rearrange("(t p r) w -> t p (r w)", t=NT, p=P, r=R)

    with tile.TileContext(nc) as tc:
        with (
            tc.tile_pool(name="io", bufs=3) as io_pool,
            tc.tile_pool(name="work", bufs=3) as work_pool,
            tc.tile_pool(name="stats", bufs=2) as stats_pool,
            tc.tile_pool(name="fp", bufs=2) as fp_pool,
            tc.psum_pool(name="ps", bufs=3) as ps_pool,
            tc.tile_pool(name="const", bufs=1) as const_pool,
        ):
            pay_t = const_pool.tile([P, W], u32, tag="pay")
            nc.gpsimd.dma_start(out=pay_t[:, :], in_=pay[:, :])
            pay_b = pay_t[:, :].unsqueeze(1).broadcast_to([P, R, W]).bitcast(f32)

            ident_t = const_pool.tile([P, P], f32, tag="idf")
            nc.gpsimd.dma_start(out=ident_t[:, :], in_=ident[:, :])
            ident_b = const_pool.tile([P, P], bf16, tag="idb")
            nc.scalar.activation(ident_b[:, :], ident_t[:, :], ACT.Copy)

            mask_t = const_pool.tile([P, 1], u32, tag="mask")
            nc.vector.memset(mask_t[:, :], PAYLOAD_MASK)
            mask_ap = mask_t[:, :1].bitcast(f32)

            res = const_pool.tile([P, NBLK * BW], f32, tag="res")

            ops = _register_custom_ops()
            pools = (io_pool, work_pool, stats_pool, fp_pool, ps_pool)
            _emit_pass(nc, pools, pred_v, targ_v, pay_b, ident_b, res, ops,
                       mask_ap)

            nc.sync.dma_start(out=out[:, :], in_=res[:, :])
    nc.compile()
    return nc


_CACHE = {}


def _run(predict, target, trace=False):
    if "nc" not in _CACHE:
        _CACHE["nc"] = _build_program()
    nc = _CACHE["nc"]

    predict = np.ascontiguousarray(np.asarray(predict, dtype=np.float32))
    target = np.ascontiguousarray(np.asarray(target, dtype=np.float32))
    payload = np.broadcast_to(
        (np.asarray(LABELS_NUM_COUNT, dtype=np.uint32) // 1000)[None, :], (P, W)
    ).copy()
    ident = np.eye(P, dtype=np.float32)

    in_maps = []
    for i in range(NCORES):
        in_maps.append(
            {
                "predict": predict[i * BS : (i + 1) * BS],
                "target": target[i * BS : (i + 1) * BS],
                "payload": payload,
                "ident": ident,
            }
        )
    res = run_bass_kernel_spmd(nc, in_maps, core_ids=list(range(NCORES)), trace=trace)
    total = np.float64(0.0)
    for r in res.results:
        total += np.float64(r["out"].astype(np.float64).sum())
    value = np.float32(total / B)
    return np.asarray(value, dtype=np.float32), res


def kernel(predict, target, penalty_matrix=None):
    value, _ = _run(predict, target, trace=False)
    return value



# revision 27
# speedup vs baseline: 1.1539x; 1.1539x over previous
"""Trainium2 Bass kernel for nn_CrossEntropyLossWeight3.

Math: per row b of predict/target [B,16]:
  probs   = softmax(predict[b])
  pre     = argmax(predict[b]);  tar = argmax(target[b])
  w       = 0 if pre==tar else penalty[tar, pre]
  loss_b  = w * probs[pre]
out = mean_b(loss_b)

Key identities used on-device:
  probs[pre]   = exp(max(x)) / sum(exp(x))      (softmax at its own argmax)
  penalty[i,j] = max(c_i,c_j)/(c_i+c_j) with distinct per-class counts c;
  with u = c[pre], v = c[tar]:  w = (u != v) * max(u,v)/(u+v).
  counts/1000 (9 bits, exact) are embedded into the low mantissa bits of the
  raw inputs, so one fused embed+segmented-max DVE scan per tensor yields
  the row max together with its argmax's class count (<= 2^-14 relative
  perturbation). Two more fused custom DVE ops evaluate the whole per-row
  weight formula straight from the embedded maxima:
    WNUM = (u!=v) * max(u,v)        SPD = u + v
  so loss_b = WNUM * exp(m) / (SPD * sumexp).

v6 engine balance (per [128, 256*16] tile; single sync HWDGE ring streams
both tensors at a measured ~428 GB/s => ~9.4us/tile of DMA):
  - DVE     : two embed+segmax f32 scans (2 x 4.4us) + WNUM/SPD/recip per
              tile (~1.3us)  => ~10.1us/tile, the critical engine
  - ACT     : exp(predict) f32->bf16 (3.7us) + exp(m) (~0.3us)
  - TensorE : row sums of E as 16 PSUM-accumulated matmuls with identity
              weights (rhs = E[:, :, w], w=0..15) -> s[p,r] lands in PSUM
              in f32, ~2-3us/tile on an otherwise idle engine
  - GPSIMD  : only the small per-tile formula mults den/num/num2/acc
              (Q7 is ~2x slower under full DMA load; it gets no streaming
              work at all)
  - DMA     : both input streams + out on the SP (sync) ring so ACT's exp
              never sits in front of a dma_start issue
  - formula : per tile, split into F1 (wn/sp/em/den/num, emitted with the
              tile) and F2 (rec/num2/acc, deferred one tile) so no engine
              head-of-line stalls on a cross-engine dependency
Sharding: pure data parallel over 8 cores (batch split); each core returns
per-partition partial sums [128,256]; host reduces and divides by B.
"""

import sys

sys.path.insert(0, "/opt/trn_rl_repo")

import numpy as np

import concourse.bass as bass
import concourse.bacc as bacc
import concourse.tile as tile
from concourse import mybir
from concourse.bass_utils import run_bass_kernel_spmd

B, W = 2097152, 16
NCORES = 8
BS = B // NCORES          # rows per core
P = 128                   # SBUF partitions
R = 256                   # rows per partition per tile
F = R * W                 # free elems per partition per tile
TILE_ROWS = P * R
NT = BS // TILE_ROWS      # tiles per core

LABELS_NUM_COUNT = [500000, 120000, 80000, 45000, 30000, 250000, 15000, 9000,
                    60000, 7000, 180000, 22000, 11000, 95000, 5000, 40000]

f32 = mybir.dt.float32
bf16 = mybir.dt.bfloat16
u32 = mybir.dt.uint32
AX = mybir.AxisListType
OP = mybir.AluOpType
ACT = mybir.ActivationFunctionType

PAYLOAD_BITS = 9          # counts/1000 <= 500 fits in 9 bits exactly
PAYLOAD_MASK = (1 << PAYLOAD_BITS) - 1
F_2P23 = 8388608.0        # bit pattern 0x4B000000; OR'ing these bits onto the
                          # 9-bit payload makes the exact float 2^23 + payload
F_2P24 = 16777216.0


def _register_custom_ops():
    """Three runtime-registered custom DVE ops.

    EMBMAX_SEG_ANT: fused "embed payload + segmented max" scan (see v2/v3
      history): body = Scan(MAX, ((x|c)^c)|pay, _subdim_step=Zero) over a
      [P, S, 16] view; stride-0 out leaves per-segment maxima in [P, S].
      The OR/XOR form avoids an AND with 0xFFFFFE00 (NaN bit pattern).
    WNUM_ANT(me, mt; s0=mask, s1=2^23): with u' = (me & mask) | bits(s1),
      v' = (mt & mask) | bits(s1)  (both exact floats 2^23 + count):
      out = (u' != v') * (max(u',v') - 2^23) = (u!=v)*max(u,v).
    SPD_ANT(me, mt; s0, s1, imm2=2^24): out = u' + v' - 2^24 = u + v.
    """
    import numpy as np_

    from concourse.dve_spec import (
        Spec, Src0, Src1, C0, C1, C2, Bin, AluOp, lower, ne, maxx, Zero,
    )
    from concourse.dve_ops import (
        DveOp,
        OPS,
        CUSTOM_DVE_SPECS,
        _SUB_OPCODE_FOR_NAME,
        _CUSTOM_DVE_ROW_BASE,
        _COMPILE_CACHE,
    )
    from concourse.dve_uop import DveOpSpec
    import concourse.dve_spec as ds

    def reg(name, spec, rd1):
        for o in OPS:
            if o.name == name:
                return o
        shas = {}
        for ver in ("v3", "v4"):
            uops = lower(spec, ver=ver)
            s = DveOpSpec(
                name=name,
                opcode=_CUSTOM_DVE_ROW_BASE + len(OPS),
                uops=uops,
                rd1_en=rd1,
            )
            shas[ver] = s.sha(ver)
        op = DveOp(name, spec, subdim=False, uops_sha=shas)
        _SUB_OPCODE_FOR_NAME[name] = _CUSTOM_DVE_ROW_BASE + len(OPS)
        OPS.append(op)
        CUSTOM_DVE_SPECS[name] = spec
        return op

    embed_expr = Bin(
        AluOp.BITWISE_OR,
        Bin(AluOp.BITWISE_XOR, Bin(AluOp.BITWISE_OR, Src0, C0), C0),
        Src1,
    )

    def _ref_embmax(in0, in1, s0, s1, imm2):
        emb = (
            ((in0.view(np_.uint32) | PAYLOAD_MASK) ^ PAYLOAD_MASK)
            | in1.view(np_.uint32)
        ).view(np_.float32)
        return np_.maximum.accumulate(emb, axis=-1)

    def reg_embmax():
        name = "EMBMAX_SEG_ANT"
        for o in OPS:
            if o.name == name:
                return o
        seg = ds.Scan(op=AluOp.MAX, expr=embed_expr, init=None, _subdim_step=Zero)
        spec = Spec(body=seg, reference=_ref_embmax)
        orig_so, orig_nas = ds._scan_overrides, ds._node_as_stage

        def patched_so(scans, node_stage):
            seed, step = {}, {}
            for scan in scans:
                d = node_stage[scan]
                init = (
                    scan.init
                    if scan.init is not None
                    else ds._ACCUM_IDENTITY[scan.op]
                )
                seed[d] = orig_nas(init)
                if scan._subdim_step is not None:
                    step[d] = ds._Stage(AluOp.BYPASS, scan.expr)
            return seed, step

        def patched_nas(e):
            if isinstance(e, ds.Scan) and e._subdim_step is not None:
                return ds._Stage(e.op, ds.AluInp.CURR_ALU_OUT, e.expr)
            return orig_nas(e)

        uops_by_ver, shas = {}, {}
        ds._scan_overrides, ds._node_as_stage = patched_so, patched_nas
        try:
            for ver in ("v3", "v4"):
                uops_by_ver[ver] = lower(spec, ver=ver)
        finally:
            ds._scan_overrides, ds._node_as_stage = orig_so, orig_nas
        opcode = _CUSTOM_DVE_ROW_BASE + len(OPS)
        for ver in ("v3", "v4"):
            s = DveOpSpec(name=name, opcode=opcode, uops=uops_by_ver[ver], rd1_en=True)
            shas[ver] = s.sha(ver)
            _COMPILE_CACHE[(name, ver)] = s
        op = DveOp(name, spec, subdim=True, uops_sha=shas)
        _SUB_OPCODE_FOR_NAME[name] = opcode
        OPS.append(op)
        CUSTOM_DVE_SPECS[name] = spec
        return op

    def _uprime(src):
        return Bin(AluOp.BITWISE_OR, Bin(AluOp.BITWISE_AND, src, C0), C1)

    def _np_uprime(x):
        return (
            (x.view(np_.uint32) & PAYLOAD_MASK) | np_.uint32(0x4B000000)
        ).view(np_.float32)

    up_e, vp_e = _uprime(Src0), _uprime(Src1)
    wnum_spec = Spec(
        body=Bin(
            AluOp.MULTIPLY,
            ne(up_e, vp_e),
            Bin(AluOp.SUBTRACT, maxx(up_e, vp_e), C1),
        ),
        reference=lambda in0, in1, s0, s1, imm2: np_.where(
            _np_uprime(in0) != _np_uprime(in1),
            np_.maximum(_np_uprime(in0), _np_uprime(in1)) - np_.float32(F_2P23),
            np_.float32(0.0),
        ).astype(np_.float32),
    )
    spd_spec = Spec(
        body=Bin(
            AluOp.SUBTRACT, Bin(AluOp.ADD, up_e, vp_e), C2
        ),
        reference=lambda in0, in1, s0, s1, imm2: (
            _np_uprime(in0) + _np_uprime(in1) - np_.float32(F_2P24)
        ).astype(np_.float32),
    )

    embed = reg_embmax()
    wnum = reg("WNUM_ANT", wnum_spec, rd1=True)
    spd = reg("SPD_ANT", spd_spec, rd1=True)
    return embed, wnum, spd


BW = 2 * R                 # formula block width: one tile PAIR (512)
NBLK = NT // 2             # formula blocks (pairs) per core


def _emit_tile(nc, pools, pred_v, targ_v, pay_b, t, embed_op, mask_ap,
               me, mt, e2):
    """Streaming part for one [128, R*16] tile. Row stats land in column
    half t%2 of the pair stats tiles me/mt; exp lands in half t%2 of the
    bf16 pair tile e2."""
    io_pool = pools[0]
    cols = slice((t % 2) * R, (t % 2 + 1) * R)

    # both input streams on the sync HWDGE ring: SP issues nothing else, so
    # dma_starts go out back-to-back and are never stuck behind an ACT op
    xp = io_pool.tile([P, F], f32, tag="xp")
    nc.sync.dma_start(out=xp[:, :], in_=pred_v[t])
    xt = io_pool.tile([P, F], f32, tag="xt")
    nc.sync.dma_start(out=xt[:, :], in_=targ_v[t])

    # fused embed + segmented max over RAW predict on DVE; runs concurrently
    # with the exp pass on ACT (both only read xp)
    xp3 = xp[:, :].rearrange("p (r w) -> p r w", w=W)
    nc.vector._custom_dve(
        embed_op,
        out=me[:, cols].unsqueeze(2).broadcast_to([P, R, W]),
        in0=xp3, in1=pay_b, s0=mask_ap,
    )

    # E = exp(predict) on ScalarE into half t%2 of the bf16 pair tile
    # (contiguous write — a transposed write ran 5x slower on ACT)
    nc.scalar.activation(e2[:, (t % 2) * F:(t % 2 + 1) * F], xp[:, :],
                         ACT.Exp)

    # target side: fused embed + segmented max on DVE
    xt3 = xt[:, :].rearrange("p (r w) -> p r w", w=W)
    nc.vector._custom_dve(
        embed_op,
        out=mt[:, cols].unsqueeze(2).broadcast_to([P, R, W]),
        in0=xt3, in1=pay_b, s0=mask_ap,
    )


def _emit_sums_pe(nc, pools, e2, half, ident_b):
    """Row sums of one e2 half (one tile) on TensorE: 16 matmuls with
    identity weights, one per class column (strided rhs -> ~2.1ns/col on
    HW), PSUM-accumulated in f32. Issued per tile (right after its exp) so
    PE work spreads across the whole stream. Returns the PSUM tile; the
    ACT drain to SBUF is emitted by the caller one tile LATER, so the next
    exp in ACT program order never waits on this tile's matmuls."""
    ps_pool = pools[4]
    s2 = ps_pool.tile([P, R], f32, tag="s2")
    e3h = e2[:, half * F:(half + 1) * F].rearrange("p (r w) -> p r w", w=W)
    for w in range(W):
        nc.tensor.matmul(
            out=s2[:, :], lhsT=ident_b[:, :], rhs=e3h[:, :, w],
            start=(w == 0), stop=(w == W - 1),
        )
    return s2


def _emit_sums_dve_tree(nc, pools, e2, half, s2c):
    """Row sums of one e2 half via the contiguous-halves bf16 pairwise-add
    tree on DVE. Used ONLY for the final tile: the PE can't start that
    tile's ~10us of matmuls until the stream has already ended, and the
    GPSIMD tree there ran ~13us while halving concurrent DVE scan speed;
    DVE's own tree is ~2.6-4.6us with no cross-engine contention."""
    work_pool = pools[1]
    e3 = e2[:, half * F:(half + 1) * F].rearrange("p (r w) -> p r w", w=W)
    l1 = work_pool.tile([P, R * 8], bf16, tag="l1")
    l1v = l1[:, :].rearrange("p (r h) -> p r h", h=8)
    nc.vector.tensor_tensor(l1v, e3[:, :, 0:8], e3[:, :, 8:16], op=OP.add)
    l2 = work_pool.tile([P, R * 4], bf16, tag="l2")
    l2v = l2[:, :].rearrange("p (r h) -> p r h", h=4)
    nc.vector.tensor_tensor(l2v, l1v[:, :, 0:4], l1v[:, :, 4:8], op=OP.add)
    l3 = work_pool.tile([P, R * 2], bf16, tag="l3")
    l3v = l3[:, :].rearrange("p (r h) -> p r h", h=2)
    nc.vector.tensor_tensor(l3v, l2v[:, :, 0:2], l2v[:, :, 2:4], op=OP.add)
    nc.vector.tensor_tensor(
        s2c[:, half * R:(half + 1) * R].unsqueeze(2),
        l3v[:, :, 0:1], l3v[:, :, 1:2], op=OP.add)


def _emit_f1(nc, pools, me, mt, s2c, ops, mask_ap, last):
    """Formula stage 1 for one [128, BW] pair block (emitted with its odd
    tile): everything that only needs me/mt/s.
      wn = (u!=v)*max(u,v)   sp = u+v   em = exp(m)
      den = sp * sumexp      num = wn * em
    den/num run on GPSIMD (near idle) except for the last block, where
    DVE's ~0.7us ops shorten the post-DMA tail. Returns (den, num)."""
    fp_pool = pools[3]
    _, wnum_op, spd_op = ops
    mul_eng = nc.vector if last else nc.gpsimd

    wn = fp_pool.tile([P, BW], f32, tag="wn")
    nc.vector._custom_dve(wnum_op, out=wn[:, :], in0=me[:, :], in1=mt[:, :],
                          s0=mask_ap, s1=F_2P23)
    sp = fp_pool.tile([P, BW], f32, tag="sp")
    nc.vector._custom_dve(spd_op, out=sp[:, :], in0=me[:, :], in1=mt[:, :],
                          s0=mask_ap, s1=F_2P23, imm2=F_2P24)
    # em = exp(m): payload bits perturb m by <= 2^-14 relative — in budget
    em = fp_pool.tile([P, BW], f32, tag="em")
    nc.scalar.activation(em[:, :], me[:, :], ACT.Exp)

    den = fp_pool.tile([P, BW], f32, tag="dn")
    if last:
        # both s2c halves are already written (tile NT-2 drained during
        # tile NT-1; tile NT-1 via the DVE tree): one full-width den
        mul_eng.tensor_tensor(den[:, :], sp[:, :], s2c[:, :], op=OP.mult)
    else:
        # s2c's second half is only drained from PSUM during the NEXT
        # tile (deferred ACT drain), so den's second half moves to F2
        mul_eng.tensor_tensor(den[:, :R], sp[:, :R], s2c[:, :R],
                              op=OP.mult)
    num = fp_pool.tile([P, BW], f32, tag="nm")
    mul_eng.tensor_tensor(num[:, :], wn[:, :], em[:, :], op=OP.mult)
    return sp, s2c, den, num


def _emit_f2(nc, pools, res_sl, sp, s2c, den, num, last):
    """Formula stage 2 (emitted one tile later so no engine head-of-line
    stalls on a cross-engine dep): finish den, then res = num / den."""
    fp_pool = pools[3]
    if not last:
        nc.gpsimd.tensor_tensor(den[:, R:], sp[:, R:], s2c[:, R:],
                                op=OP.mult)
    rec = fp_pool.tile([P, BW], f32, tag="rc")
    nc.vector.reciprocal_approx_fast(out=rec[:, :], in_=den[:, :])
    mul_eng = nc.vector if last else nc.gpsimd
    mul_eng.tensor_tensor(res_sl, num[:, :], rec[:, :], op=OP.mult)


def _emit_pass(nc, pools, pred_v, targ_v, pay_b, ident_b, res, ops, mask_ap):
    _, work_pool, stats_pool, fp_pool, ps_pool, lastp_pool = pools
    embed_op = ops[0]
    pend = None            # (den, num) of the previous pair block
    pend_drain = None      # (psum_tile, s2c, half) not yet drained by ACT
    for k in range(NBLK):
        me = stats_pool.tile([P, BW], f32, tag="me")
        mt = stats_pool.tile([P, BW], f32, tag="mt")
        s2c = fp_pool.tile([P, BW], bf16, tag="s2c")
        # the LAST pair gets a dedicated e2 tile: with the shared 2-buf
        # rotation, exp(NT-1) stalls ~9us waiting for pair NBLK-3's
        # matmuls to release the buffer, right at the stream end where
        # it puts the whole DVE tail on hold. A dedicated tile adds
        # 16KB/partition but zero mid-run concurrency.
        if k == NBLK - 1:
            e2 = lastp_pool.tile([P, 2 * F], bf16, tag="e2last")
        else:
            e2 = work_pool.tile([P, 2 * F], bf16, tag="e2")
        for sub in range(2):
            t = 2 * k + sub
            _emit_tile(nc, pools, pred_v, targ_v, pay_b, t, embed_op,
                       mask_ap, me, mt, e2)
            # drain the PREVIOUS tile's PSUM sums now — after this tile's
            # exp in ACT program order, so exp never waits on matmuls
            if pend_drain is not None:
                ps, dst, dhalf = pend_drain
                nc.scalar.activation(dst[:, dhalf * R:(dhalf + 1) * R],
                                     ps[:, :], ACT.Copy)
                pend_drain = None
            if t < NT - 1:
                pend_drain = (_emit_sums_pe(nc, pools, e2, sub, ident_b),
                              s2c, sub)
            else:
                _emit_sums_dve_tree(nc, pools, e2, sub, s2c)
            # interleave the previous pair's F2 early in this pair
            if pend is not None and sub == 0:
                _emit_f2(nc, pools, res[:, (k - 1) * BW:k * BW], *pend,
                         last=False)
                pend = None
        pend = _emit_f1(nc, pools, me, mt, s2c, ops, mask_ap,
                        last=(k == NBLK - 1))
    _emit_f2(nc, pools, res[:, (NBLK - 1) * BW:], *pend, last=True)


def _build_program():
    nc = bacc.Bacc("TRN2", target_bir_lowering=False, debug=False)
    pred = nc.dram_tensor("predict", [BS, W], f32, kind="ExternalInput")
    targ = nc.dram_tensor("target", [BS, W], f32, kind="ExternalInput")
    pay = nc.dram_tensor("payload", [P, W], u32, kind="ExternalInput")
    ident = nc.dram_tensor("ident", [P, P], f32, kind="ExternalInput")
    out = nc.dram_tensor("out", [P, NBLK * BW], f32, kind="ExternalOutput")

    pred_v = pred[:, :].rearrange("(t p r) w -> t p (r w)", t=NT, p=P, r=R)
    targ_v = targ[:, :].rearrange("(t p r) w -> t p (r w)", t=NT, p=P, r=R)

    with tile.TileContext(nc) as tc:
        with (
            tc.tile_pool(name="io", bufs=3) as io_pool,
            tc.tile_pool(name="work", bufs=2) as work_pool,
            tc.tile_pool(name="stats", bufs=2) as stats_pool,
            tc.tile_pool(name="fp", bufs=2) as fp_pool,
            tc.psum_pool(name="ps", bufs=3) as ps_pool,
            tc.tile_pool(name="lastp", bufs=1) as lastp_pool,
            tc.tile_pool(name="const", bufs=1) as const_pool,
        ):
            pay_t = const_pool.tile([P, W], u32, tag="pay")
            nc.gpsimd.dma_start(out=pay_t[:, :], in_=pay[:, :])
            pay_b = pay_t[:, :].unsqueeze(1).broadcast_to([P, R, W]).bitcast(f32)

            ident_t = const_pool.tile([P, P], f32, tag="idf")
            nc.gpsimd.dma_start(out=ident_t[:, :], in_=ident[:, :])
            ident_b = const_pool.tile([P, P], bf16, tag="idb")
            nc.scalar.activation(ident_b[:, :], ident_t[:, :], ACT.Copy)

            mask_t = const_pool.tile([P, 1], u32, tag="mask")
            nc.vector.memset(mask_t[:, :], PAYLOAD_MASK)
            mask_ap = mask_t[:, :1].bitcast(f32)

            res = const_pool.tile([P, NBLK * BW], f32, tag="res")

            ops = _register_custom_ops()
            pools = (io_pool, work_pool, stats_pool, fp_pool, ps_pool, lastp_pool)
            _emit_pass(nc, pools, pred_v, targ_v, pay_b, ident_b, res, ops,
                       mask_ap)

            nc.sync.dma_start(out=out[:, :], in_=res[:, :])
    nc.compile()
    return nc


_CACHE = {}


def _run(predict, target, trace=False):
    if "nc" not in _CACHE:
        _CACHE["nc"] = _build_program()
    nc = _CACHE["nc"]

    predict = np.ascontiguousarray(np.asarray(predict, dtype=np.float32))
    target = np.ascontiguousarray(np.asarray(target, dtype=np.float32))
    payload = np.broadcast_to(
        (np.asarray(LABELS_NUM_COUNT, dtype=np.uint32) // 1000)[None, :], (P, W)
    ).copy()
    ident = np.eye(P, dtype=np.float32)

    in_maps = []
    for i in range(NCORES):
        in_maps.append(
            {
                "predict": predict[i * BS : (i + 1) * BS],
                "target": target[i * BS : (i + 1) * BS],
                "payload": payload,
                "ident": ident,
            }
        )
    res = run_bass_kernel_spmd(nc, in_maps, core_ids=list(range(NCORES)), trace=trace)
    total = np.float64(0.0)
    for r in res.results:
        total += np.float64(r["out"].astype(np.float64).sum())
    value = np.float32(total / B)
    return np.asarray(value, dtype=np.float32), res


def kernel(predict, target, penalty_matrix=None):
    value, _ = _run(predict, target, trace=False)
    return value



# revision 28
# speedup vs baseline: 1.1629x; 1.0079x over previous
"""Trainium2 Bass kernel for nn_CrossEntropyLossWeight3.

Math: per row b of predict/target [B,16]:
  probs   = softmax(predict[b])
  pre     = argmax(predict[b]);  tar = argmax(target[b])
  w       = 0 if pre==tar else penalty[tar, pre]
  loss_b  = w * probs[pre]
out = mean_b(loss_b)

Key identities used on-device:
  probs[pre]   = exp(max(x)) / sum(exp(x))      (softmax at its own argmax)
  penalty[i,j] = max(c_i,c_j)/(c_i+c_j) with distinct per-class counts c;
  with u = c[pre], v = c[tar]:  w = (u != v) * max(u,v)/(u+v).
  counts/1000 (9 bits, exact) are embedded into the low mantissa bits of the
  raw inputs, so one fused embed+segmented-max DVE scan per tensor yields
  the row max together with its argmax's class count (<= 2^-14 relative
  perturbation). Two more fused custom DVE ops evaluate the whole per-row
  weight formula straight from the embedded maxima:
    WNUM = (u!=v) * max(u,v)        SPD = u + v
  so loss_b = WNUM * exp(m) / (SPD * sumexp).

v6 engine balance (per [128, 256*16] tile; single sync HWDGE ring streams
both tensors at a measured ~428 GB/s => ~9.4us/tile of DMA):
  - DVE     : two embed+segmax f32 scans (2 x 4.4us) + WNUM/SPD/recip per
              tile (~1.3us)  => ~10.1us/tile, the critical engine
  - ACT     : exp(predict) f32->bf16 (3.7us) + exp(m) (~0.3us)
  - TensorE : row sums of E as 16 PSUM-accumulated matmuls with identity
              weights (rhs = E[:, :, w], w=0..15) -> s[p,r] lands in PSUM
              in f32, ~2-3us/tile on an otherwise idle engine
  - GPSIMD  : only the small per-tile formula mults den/num/num2/acc
              (Q7 is ~2x slower under full DMA load; it gets no streaming
              work at all)
  - DMA     : both input streams + out on the SP (sync) ring so ACT's exp
              never sits in front of a dma_start issue
  - formula : per tile, split into F1 (wn/sp/em/den/num, emitted with the
              tile) and F2 (rec/num2/acc, deferred one tile) so no engine
              head-of-line stalls on a cross-engine dependency
Sharding: pure data parallel over 8 cores (batch split); each core returns
per-partition partial sums [128,256]; host reduces and divides by B.
"""

import sys

sys.path.insert(0, "/opt/trn_rl_repo")

import numpy as np

import concourse.bass as bass
import concourse.bacc as bacc
import concourse.tile as tile
from concourse import mybir
from concourse.bass_utils import run_bass_kernel_spmd

B, W = 2097152, 16
NCORES = 8
BS = B // NCORES          # rows per core
P = 128                   # SBUF partitions
R = 256                   # rows per partition per tile
F = R * W                 # free elems per partition per tile
TILE_ROWS = P * R
NT = BS // TILE_ROWS      # tiles per core

LABELS_NUM_COUNT = [500000, 120000, 80000, 45000, 30000, 250000, 15000, 9000,
                    60000, 7000, 180000, 22000, 11000, 95000, 5000, 40000]

f32 = mybir.dt.float32
bf16 = mybir.dt.bfloat16
u32 = mybir.dt.uint32
AX = mybir.AxisListType
OP = mybir.AluOpType
ACT = mybir.ActivationFunctionType

PAYLOAD_BITS = 9          # counts/1000 <= 500 fits in 9 bits exactly
PAYLOAD_MASK = (1 << PAYLOAD_BITS) - 1
F_2P23 = 8388608.0        # bit pattern 0x4B000000; OR'ing these bits onto the
                          # 9-bit payload makes the exact float 2^23 + payload
F_2P24 = 16777216.0


def _register_custom_ops():
    """Three runtime-registered custom DVE ops.

    EMBMAX_SEG_ANT: fused "embed payload + segmented max" scan (see v2/v3
      history): body = Scan(MAX, ((x|c)^c)|pay, _subdim_step=Zero) over a
      [P, S, 16] view; stride-0 out leaves per-segment maxima in [P, S].
      The OR/XOR form avoids an AND with 0xFFFFFE00 (NaN bit pattern).
    WNUM_ANT(me, mt; s0=mask, s1=2^23): with u' = (me & mask) | bits(s1),
      v' = (mt & mask) | bits(s1)  (both exact floats 2^23 + count):
      out = (u' != v') * (max(u',v') - 2^23) = (u!=v)*max(u,v).
    SPD_ANT(me, mt; s0, s1, imm2=2^24): out = u' + v' - 2^24 = u + v.
    """
    import numpy as np_

    from concourse.dve_spec import (
        Spec, Src0, Src1, C0, C1, C2, Bin, AluOp, lower, ne, maxx, Zero,
    )
    from concourse.dve_ops import (
        DveOp,
        OPS,
        CUSTOM_DVE_SPECS,
        _SUB_OPCODE_FOR_NAME,
        _CUSTOM_DVE_ROW_BASE,
        _COMPILE_CACHE,
    )
    from concourse.dve_uop import DveOpSpec
    import concourse.dve_spec as ds

    def reg(name, spec, rd1):
        for o in OPS:
            if o.name == name:
                return o
        shas = {}
        for ver in ("v3", "v4"):
            uops = lower(spec, ver=ver)
            s = DveOpSpec(
                name=name,
                opcode=_CUSTOM_DVE_ROW_BASE + len(OPS),
                uops=uops,
                rd1_en=rd1,
            )
            shas[ver] = s.sha(ver)
        op = DveOp(name, spec, subdim=False, uops_sha=shas)
        _SUB_OPCODE_FOR_NAME[name] = _CUSTOM_DVE_ROW_BASE + len(OPS)
        OPS.append(op)
        CUSTOM_DVE_SPECS[name] = spec
        return op

    embed_expr = Bin(
        AluOp.BITWISE_OR,
        Bin(AluOp.BITWISE_XOR, Bin(AluOp.BITWISE_OR, Src0, C0), C0),
        Src1,
    )

    def _ref_embmax(in0, in1, s0, s1, imm2):
        emb = (
            ((in0.view(np_.uint32) | PAYLOAD_MASK) ^ PAYLOAD_MASK)
            | in1.view(np_.uint32)
        ).view(np_.float32)
        return np_.maximum.accumulate(emb, axis=-1)

    def reg_embmax():
        name = "EMBMAX_SEG_ANT"
        for o in OPS:
            if o.name == name:
                return o
        seg = ds.Scan(op=AluOp.MAX, expr=embed_expr, init=None, _subdim_step=Zero)
        spec = Spec(body=seg, reference=_ref_embmax)
        orig_so, orig_nas = ds._scan_overrides, ds._node_as_stage

        def patched_so(scans, node_stage):
            seed, step = {}, {}
            for scan in scans:
                d = node_stage[scan]
                init = (
                    scan.init
                    if scan.init is not None
                    else ds._ACCUM_IDENTITY[scan.op]
                )
                seed[d] = orig_nas(init)
                if scan._subdim_step is not None:
                    step[d] = ds._Stage(AluOp.BYPASS, scan.expr)
            return seed, step

        def patched_nas(e):
            if isinstance(e, ds.Scan) and e._subdim_step is not None:
                return ds._Stage(e.op, ds.AluInp.CURR_ALU_OUT, e.expr)
            return orig_nas(e)

        uops_by_ver, shas = {}, {}
        ds._scan_overrides, ds._node_as_stage = patched_so, patched_nas
        try:
            for ver in ("v3", "v4"):
                uops_by_ver[ver] = lower(spec, ver=ver)
        finally:
            ds._scan_overrides, ds._node_as_stage = orig_so, orig_nas
        opcode = _CUSTOM_DVE_ROW_BASE + len(OPS)
        for ver in ("v3", "v4"):
            s = DveOpSpec(name=name, opcode=opcode, uops=uops_by_ver[ver], rd1_en=True)
            shas[ver] = s.sha(ver)
            _COMPILE_CACHE[(name, ver)] = s
        op = DveOp(name, spec, subdim=True, uops_sha=shas)
        _SUB_OPCODE_FOR_NAME[name] = opcode
        OPS.append(op)
        CUSTOM_DVE_SPECS[name] = spec
        return op

    def _uprime(src):
        return Bin(AluOp.BITWISE_OR, Bin(AluOp.BITWISE_AND, src, C0), C1)

    def _np_uprime(x):
        return (
            (x.view(np_.uint32) & PAYLOAD_MASK) | np_.uint32(0x4B000000)
        ).view(np_.float32)

    up_e, vp_e = _uprime(Src0), _uprime(Src1)
    wnum_spec = Spec(
        body=Bin(
            AluOp.MULTIPLY,
            ne(up_e, vp_e),
            Bin(AluOp.SUBTRACT, maxx(up_e, vp_e), C1),
        ),
        reference=lambda in0, in1, s0, s1, imm2: np_.where(
            _np_uprime(in0) != _np_uprime(in1),
            np_.maximum(_np_uprime(in0), _np_uprime(in1)) - np_.float32(F_2P23),
            np_.float32(0.0),
        ).astype(np_.float32),
    )
    spd_spec = Spec(
        body=Bin(
            AluOp.SUBTRACT, Bin(AluOp.ADD, up_e, vp_e), C2
        ),
        reference=lambda in0, in1, s0, s1, imm2: (
            _np_uprime(in0) + _np_uprime(in1) - np_.float32(F_2P24)
        ).astype(np_.float32),
    )

    embed = reg_embmax()
    wnum = reg("WNUM_ANT", wnum_spec, rd1=True)
    spd = reg("SPD_ANT", spd_spec, rd1=True)
    return embed, wnum, spd


BW = 2 * R                 # formula block width: one tile PAIR (512)
NBLK = NT // 2             # formula blocks (pairs) per core


def _emit_tile(nc, pools, pred_v, targ_v, pay_b, t, embed_op, mask_ap,
               me, mt, e2):
    """Streaming part for one [128, R*16] tile. Row stats land in column
    half t%2 of the pair stats tiles me/mt; exp lands in half t%2 of the
    bf16 pair tile e2."""
    io_pool = pools[0]
    cols = slice((t % 2) * R, (t % 2 + 1) * R)

    # both input streams on the sync HWDGE ring: SP issues nothing else, so
    # dma_starts go out back-to-back and are never stuck behind an ACT op
    xp = io_pool.tile([P, F], f32, tag="xp")
    nc.sync.dma_start(out=xp[:, :], in_=pred_v[t])
    xt = io_pool.tile([P, F], f32, tag="xt")
    nc.sync.dma_start(out=xt[:, :], in_=targ_v[t])

    # fused embed + segmented max over RAW predict on DVE; runs concurrently
    # with the exp pass on ACT (both only read xp)
    xp3 = xp[:, :].rearrange("p (r w) -> p r w", w=W)
    nc.vector._custom_dve(
        embed_op,
        out=me[:, cols].unsqueeze(2).broadcast_to([P, R, W]),
        in0=xp3, in1=pay_b, s0=mask_ap,
    )

    # E = exp(predict) on ScalarE into half t%2 of the bf16 pair tile
    # (contiguous write — a transposed write ran 5x slower on ACT)
    nc.scalar.activation(e2[:, (t % 2) * F:(t % 2 + 1) * F], xp[:, :],
                         ACT.Exp)

    # target side: fused embed + segmented max on DVE
    xt3 = xt[:, :].rearrange("p (r w) -> p r w", w=W)
    nc.vector._custom_dve(
        embed_op,
        out=mt[:, cols].unsqueeze(2).broadcast_to([P, R, W]),
        in0=xt3, in1=pay_b, s0=mask_ap,
    )


def _emit_sums_pe(nc, pools, e2, half, ident_b):
    """Row sums of one e2 half (one tile) on TensorE: 16 matmuls with
    identity weights, one per class column (strided rhs -> ~2.1ns/col on
    HW), PSUM-accumulated in f32. Issued per tile (right after its exp) so
    PE work spreads across the whole stream. Returns the PSUM tile; the
    ACT drain to SBUF is emitted by the caller one tile LATER, so the next
    exp in ACT program order never waits on this tile's matmuls."""
    ps_pool = pools[4]
    s2 = ps_pool.tile([P, R], f32, tag="s2")
    e3h = e2[:, half * F:(half + 1) * F].rearrange("p (r w) -> p r w", w=W)
    for w in range(W):
        nc.tensor.matmul(
            out=s2[:, :], lhsT=ident_b[:, :], rhs=e3h[:, :, w],
            start=(w == 0), stop=(w == W - 1),
        )
    return s2


def _emit_sums_dve_tree(nc, pools, e2, half, s2c):
    """Row sums of one e2 half via the contiguous-halves bf16 pairwise-add
    tree on DVE. Used ONLY for the final tile: the PE can't start that
    tile's ~10us of matmuls until the stream has already ended, and the
    GPSIMD tree there ran ~13us while halving concurrent DVE scan speed;
    DVE's own tree is ~2.6-4.6us with no cross-engine contention."""
    work_pool = pools[1]
    e3 = e2[:, half * F:(half + 1) * F].rearrange("p (r w) -> p r w", w=W)
    l1 = work_pool.tile([P, R * 8], bf16, tag="l1")
    l1v = l1[:, :].rearrange("p (r h) -> p r h", h=8)
    nc.vector.tensor_tensor(l1v, e3[:, :, 0:8], e3[:, :, 8:16], op=OP.add)
    l2 = work_pool.tile([P, R * 4], bf16, tag="l2")
    l2v = l2[:, :].rearrange("p (r h) -> p r h", h=4)
    nc.vector.tensor_tensor(l2v, l1v[:, :, 0:4], l1v[:, :, 4:8], op=OP.add)
    l3 = work_pool.tile([P, R * 2], bf16, tag="l3")
    l3v = l3[:, :].rearrange("p (r h) -> p r h", h=2)
    nc.vector.tensor_tensor(l3v, l2v[:, :, 0:2], l2v[:, :, 2:4], op=OP.add)
    nc.vector.tensor_tensor(
        s2c[:, half * R:(half + 1) * R].unsqueeze(2),
        l3v[:, :, 0:1], l3v[:, :, 1:2], op=OP.add)


def _emit_f1(nc, pools, me, mt, s2c, ops, mask_ap, last):
    """Formula stage 1 for one [128, BW] pair block (emitted with its odd
    tile): everything that only needs me/mt/s.
      wn = (u!=v)*max(u,v)   sp = u+v   em = exp(m)
      den = sp * sumexp      num = wn * em
    den/num run on GPSIMD (near idle) except for the last block, where
    DVE's ~0.7us ops shorten the post-DMA tail. Returns (den, num)."""
    fp_pool = pools[3]
    _, wnum_op, spd_op = ops
    mul_eng = nc.vector if last else nc.gpsimd

    wn = fp_pool.tile([P, BW], f32, tag="wn")
    nc.vector._custom_dve(wnum_op, out=wn[:, :], in0=me[:, :], in1=mt[:, :],
                          s0=mask_ap, s1=F_2P23)
    sp = fp_pool.tile([P, BW], f32, tag="sp")
    nc.vector._custom_dve(spd_op, out=sp[:, :], in0=me[:, :], in1=mt[:, :],
                          s0=mask_ap, s1=F_2P23, imm2=F_2P24)
    # em = exp(m): payload bits perturb m by <= 2^-14 relative — in budget
    em = fp_pool.tile([P, BW], f32, tag="em")
    nc.scalar.activation(em[:, :], me[:, :], ACT.Exp)

    den = fp_pool.tile([P, BW], f32, tag="dn")
    if last:
        # both s2c halves are already written (tile NT-2 drained during
        # tile NT-1; tile NT-1 via the DVE tree): one full-width den
        mul_eng.tensor_tensor(den[:, :], sp[:, :], s2c[:, :], op=OP.mult)
    else:
        # s2c's second half is only drained from PSUM during the NEXT
        # tile (deferred ACT drain), so den's second half moves to F2
        mul_eng.tensor_tensor(den[:, :R], sp[:, :R], s2c[:, :R],
                              op=OP.mult)
    num = fp_pool.tile([P, BW], f32, tag="nm")
    mul_eng.tensor_tensor(num[:, :], wn[:, :], em[:, :], op=OP.mult)
    return sp, s2c, den, num


def _emit_f2(nc, pools, res_sl, sp, s2c, den, num, last):
    """Formula stage 2 (emitted one tile later so no engine head-of-line
    stalls on a cross-engine dep): finish den, then res = num / den."""
    fp_pool = pools[3]
    if not last:
        nc.gpsimd.tensor_tensor(den[:, R:], sp[:, R:], s2c[:, R:],
                                op=OP.mult)
    rec = fp_pool.tile([P, BW], f32, tag="rc")
    nc.vector.reciprocal_approx_fast(out=rec[:, :], in_=den[:, :])
    mul_eng = nc.vector if last else nc.gpsimd
    mul_eng.tensor_tensor(res_sl, num[:, :], rec[:, :], op=OP.mult)


def _emit_pass(nc, pools, pred_v, targ_v, pay_b, ident_b, res, ops, mask_ap):
    _, work_pool, stats_pool, fp_pool, ps_pool, lastp_pool = pools
    embed_op = ops[0]
    pend = None            # (den, num) of the previous pair block
    pend_drain = None      # (psum_tile, s2c, half) not yet drained by ACT
    for k in range(NBLK):
        me = stats_pool.tile([P, BW], f32, tag="me")
        mt = stats_pool.tile([P, BW], f32, tag="mt")
        s2c = fp_pool.tile([P, BW], bf16, tag="s2c")
        e2 = work_pool.tile([P, 2 * F], bf16, tag="e2")
        for sub in range(2):
            t = 2 * k + sub
            _emit_tile(nc, pools, pred_v, targ_v, pay_b, t, embed_op,
                       mask_ap, me, mt, e2)
            # drain the PREVIOUS tile's PSUM sums now — after this tile's
            # exp in ACT program order, so exp never waits on matmuls
            if pend_drain is not None:
                ps, dst, dhalf = pend_drain
                nc.scalar.activation(dst[:, dhalf * R:(dhalf + 1) * R],
                                     ps[:, :], ACT.Copy)
                pend_drain = None
            if t < NT - 1:
                pend_drain = (_emit_sums_pe(nc, pools, e2, sub, ident_b),
                              s2c, sub)
            else:
                _emit_sums_dve_tree(nc, pools, e2, sub, s2c)
            # interleave the previous pair's F2 early in this pair
            if pend is not None and sub == 0:
                _emit_f2(nc, pools, res[:, (k - 1) * BW:k * BW], *pend,
                         last=False)
                pend = None
        pend = _emit_f1(nc, pools, me, mt, s2c, ops, mask_ap,
                        last=(k == NBLK - 1))
    _emit_f2(nc, pools, res[:, (NBLK - 1) * BW:], *pend, last=True)


def _build_program():
    nc = bacc.Bacc("TRN2", target_bir_lowering=False, debug=False)
    pred = nc.dram_tensor("predict", [BS, W], f32, kind="ExternalInput")
    targ = nc.dram_tensor("target", [BS, W], f32, kind="ExternalInput")
    pay = nc.dram_tensor("payload", [P, W], u32, kind="ExternalInput")
    ident = nc.dram_tensor("ident", [P, P], f32, kind="ExternalInput")
    out = nc.dram_tensor("out", [P, NBLK * BW], f32, kind="ExternalOutput")

    pred_v = pred[:, :].rearrange("(t p r) w -> t p (r w)", t=NT, p=P, r=R)
    targ_v = targ[:, :].rearrange("(t p r) w -> t p (r w)", t=NT, p=P, r=R)

    with tile.TileContext(nc) as tc:
        with (
            tc.tile_pool(name="io", bufs=3) as io_pool,
            tc.tile_pool(name="work", bufs=2) as work_pool,
            tc.tile_pool(name="stats", bufs=2) as stats_pool,
            tc.tile_pool(name="fp", bufs=2) as fp_pool,
            tc.psum_pool(name="ps", bufs=3) as ps_pool,
            tc.tile_pool(name="lastp", bufs=1) as lastp_pool,
            tc.tile_pool(name="const", bufs=1) as const_pool,
        ):
            pay_t = const_pool.tile([P, W], u32, tag="pay")
            nc.gpsimd.dma_start(out=pay_t[:, :], in_=pay[:, :])
            pay_b = pay_t[:, :].unsqueeze(1).broadcast_to([P, R, W]).bitcast(f32)

            ident_t = const_pool.tile([P, P], f32, tag="idf")
            nc.gpsimd.dma_start(out=ident_t[:, :], in_=ident[:, :])
            ident_b = const_pool.tile([P, P], bf16, tag="idb")
            nc.scalar.activation(ident_b[:, :], ident_t[:, :], ACT.Copy)

            mask_t = const_pool.tile([P, 1], u32, tag="mask")
            nc.vector.memset(mask_t[:, :], PAYLOAD_MASK)
            mask_ap = mask_t[:, :1].bitcast(f32)

            res = const_pool.tile([P, NBLK * BW], f32, tag="res")

            ops = _register_custom_ops()
            pools = (io_pool, work_pool, stats_pool, fp_pool, ps_pool, lastp_pool)
            _emit_pass(nc, pools, pred_v, targ_v, pay_b, ident_b, res, ops,
                       mask_ap)

            nc.sync.dma_start(out=out[:, :], in_=res[:, :])
    nc.compile()
    return nc


_CACHE = {}


def _run(predict, target, trace=False):
    if "nc" not in _CACHE:
        _CACHE["nc"] = _build_program()
    nc = _CACHE["nc"]

    predict = np.ascontiguousarray(np.asarray(predict, dtype=np.float32))
    target = np.ascontiguousarray(np.asarray(target, dtype=np.float32))
    payload = np.broadcast_to(
        (np.asarray(LABELS_NUM_COUNT, dtype=np.uint32) // 1000)[None, :], (P, W)
    ).copy()
    ident = np.eye(P, dtype=np.float32)

    in_maps = []
    for i in range(NCORES):
        in_maps.append(
            {
                "predict": predict[i * BS : (i + 1) * BS],
                "target": target[i * BS : (i + 1) * BS],
                "payload": payload,
                "ident": ident,
            }
        )
    res = run_bass_kernel_spmd(nc, in_maps, core_ids=list(range(NCORES)), trace=trace)
    total = np.float64(0.0)
    for r in res.results:
        total += np.float64(r["out"].astype(np.float64).sum())
    value = np.float32(total / B)
    return np.asarray(value, dtype=np.float32), res


def kernel(predict, target, penalty_matrix=None):
    value, _ = _run(predict, target, trace=False)
    return value

